# revision 9
# baseline (speedup 1.0000x reference)
"""Trainium2 Bass kernel for nn_ACArLModule (gnn_message_passing).

Strategy: data-parallel over batch B=8 across 8 NeuronCores. Per core:
  - f_global mean + two small MLPs (z path)
  - affinity MLP factored: layer1 = gather-free per-point projections
    (F_pts @ W1a/b + 0.5*coordemb @ W1c) combined via static shifted-slice
    adds over the 12 neighbor offsets of the 16x16 sampled grid
  - A output written sparsely (diag + 2756 vals per class); the runtime
    zero-fills output buffers
  - diffusion on the 256 sampled points only (other rows are exact no-ops)
"""
import sys
import numpy as np

sys.path.insert(0, '/opt/trn_rl_repo')

import ml_dtypes

BF16 = ml_dtypes.bfloat16

# ---- static structure ----
B, D, H, W = 8, 768, 32, 32
C, K = 2, 8
HW = H * W
GN = 16          # sampled grid is 16x16 (stride 2)
P_FULL = 12 * 256  # offset-blocked pair slots (2756 valid of 3072)

OFFS = [(-2, 0), (-1, -1), (-1, 0), (-1, 1), (0, -2), (0, -1),
        (0, 1), (0, 2), (1, -1), (1, 0), (1, 1), (2, 0)]


def _rect(di, dj):
    i0, i1 = max(0, -di), GN - max(0, di)
    j0, j1 = max(0, -dj), GN - max(0, dj)
    return i0, i1, j0, j1


def _host_consts():
    mask = np.zeros((24, 256), np.float32)      # lane = 2*o + c
    for o, (di, dj) in enumerate(OFFS):
        i0, i1, j0, j1 = _rect(di, dj)
        m = np.zeros((GN, GN), np.float32)
        m[i0:i1, j0:j1] = 1.0
        mask[2 * o + 0] = m.ravel()
        mask[2 * o + 1] = m.ravel()
    cma = np.zeros((24, 2), np.float32)
    for lane in range(24):
        cma[lane, lane % 2] = 1.0
    cmb = np.ascontiguousarray(cma.T)
    return mask, cma, cmb


_BUILD = None


def _build():
    global _BUILD
    if _BUILD is not None:
        return _BUILD
    import concourse.bass as bass
    import concourse.tile as tile
    from concourse import bacc, mybir
    import bass_rust
    from contextlib import ExitStack

    f32 = mybir.dt.float32
    bf16 = mybir.dt.bfloat16
    AX = mybir.AxisListType.X
    AF = mybir.ActivationFunctionType
    OP = mybir.AluOpType

    nc = bacc.Bacc("TRN2", target_bir_lowering=False, debug=False, num_devices=8)

    dp = nc.declare_dram_parameter
    feat = dp("feat", [768, 1024], bf16, isOutput=False)
    w1 = dp("w1", [2, 1664, 256], bf16, isOutput=False)
    w2 = dp("w2", [2, 256, 128], bf16, isOutput=False)
    w3 = dp("w3", [128, 2], bf16, isOutput=False)
    e_in = dp("e", [128, 256], bf16, isOutput=False)
    mw1 = dp("mw1", [768, 256], bf16, isOutput=False)
    lw1 = dp("lw1", [768, 256], bf16, isOutput=False)
    mw2 = dp("mw2", [128, 16], bf16, isOutput=False)
    lw2 = dp("lw2", [128, 16], bf16, isOutput=False)
    b1 = dp("b1", [128, 4], f32, isOutput=False)
    b2 = dp("b2", [128, 2], f32, isOutput=False)
    b3 = dp("b3", [1, 2], f32, isOutput=False)
    mb1 = dp("mb1", [128, 2], f32, isOutput=False)
    lb1 = dp("lb1", [128, 2], f32, isOutput=False)
    mb2 = dp("mb2", [8, 1], f32, isOutput=False)
    lb2 = dp("lb2", [8, 1], f32, isOutput=False)
    cam = dp("cam", [2, 1024], f32, isOutput=False)
    eps_in = dp("eps", [8, 1], f32, isOutput=False)
    conf_in = dp("conf", [2, 2], f32, isOutput=False)
    mask_in = dp("mask", [24, 256], f32, isOutput=False)
    cma_in = dp("cma", [24, 2], f32, isOutput=False)
    cmb_in = dp("cmb", [2, 24], f32, isOutput=False)

    A_out = dp("A", [2, 1024, 1024], f32, isOutput=True)
    cam_o = dp("cam_o", [2, 1024], f32, isOutput=True)
    conf_o = dp("conf_o", [2, 1024], f32, isOutput=True)
    z_o = dp("z_o", [3, 8], f32, isOutput=True)

    A_flat_t = A_out[:].rearrange("c a b -> (c a b)").tensor

    def grid5(ap2d):
        # (p, 1024) -> strided sampled (p, 16, 16)
        return ap2d.rearrange("p (i a j b) -> p i a j b", i=16, a=2, j=16, b=2)[:, :, 0, :, 0]

    with tile.TileContext(nc) as tc, ExitStack() as ctx:
        per = ctx.enter_context(tc.tile_pool(name="per", bufs=1))
        pps = ctx.enter_context(tc.tile_pool(name="pps", bufs=1, space="PSUM"))
        pg = px2 = pl3 = pzz = ptail = pps
        pm = ctx.enter_context(tc.tile_pool(name="pm", bufs=2))
        pdram = ctx.enter_context(tc.tile_pool(name="pdram", bufs=1, space="DRAM"))

        T = lambda shape, dt, tag: per.tile(list(shape), dt, tag=tag, name=tag)

        # persistent SBUF
        f_sb = T((128, 6144), bf16, "f_sb")
        fp_sb = T((128, 1536), bf16, "fp_sb")
        w1_sb = T((128, 6656), bf16, "w1_sb")
        w2_sb = T((128, 512), bf16, "w2_sb")
        w3_sb = T((128, 2), bf16, "w3_sb")
        e_sb = T((128, 256), bf16, "e_sb")
        mw1_sb = T((128, 1536), bf16, "mw1_sb")
        lw1_sb = T((128, 1536), bf16, "lw1_sb")
        mw2_sb = T((128, 16), bf16, "mw2_sb")
        lw2_sb = T((128, 16), bf16, "lw2_sb")
        b1_sb = T((128, 4), f32, "b1_sb")
        b2_sb = T((128, 2), f32, "b2_sb")
        b3_sb = T((1, 2), f32, "b3_sb")
        mb1_sb = T((128, 2), f32, "mb1_sb")
        lb1_sb = T((128, 2), f32, "lb1_sb")
        mb2_sb = T((8, 1), f32, "mb2_sb")
        lb2_sb = T((8, 1), f32, "lb2_sb")
        cam_sb = T((2, 1024), f32, "cam_sb")
        eps_sb = T((8, 1), f32, "eps_sb")
        conf_sb = T((2, 2), f32, "conf_sb")
        mask_sb = T((24, 256), f32, "mask_sb")
        cma_sb = T((24, 2), f32, "cma_sb")
        cmb_sb = T((2, 24), f32, "cmb_sb")

        x1 = [[T((128, 3072), bf16, f"x1_{c}_{ht}") for ht in range(2)] for c in range(2)]
        x2 = [T((128, 3072), bf16, f"x2_{c}") for c in range(2)]
        pa = [[T((128, 256), bf16, f"pa_{c}_{ht}") for ht in range(2)] for c in range(2)]
        pb = [[T((128, 256), bf16, f"pb_{c}_{ht}") for ht in range(2)] for c in range(2)]
        fsum = T((128, 6), f32, "fsum")
        fsum_bf = T((128, 6), bf16, "fsum_bf")
        scratch = T((128, 1024), bf16, "scratch")
        h1m = T((128, 2), bf16, "h1m")
        h1l = T((128, 2), bf16, "h1l")
        zmu_sb = T((8, 1), f32, "zmu_sb")
        zls_sb = T((8, 1), f32, "zls_sb")
        ez_sb = T((8, 1), f32, "ez_sb")
        zs_sb = T((8, 1), f32, "zs_sb")
        val_sb = T((1, 6144), f32, "val_sb")   # [c*3072 + o*256 + cell]
        val24 = T((24, 256), f32, "val24")
        valm = T((24, 256), f32, "valm")
        valp = T((24, 256), f32, "valp")
        s2_sb = T((2, 256), f32, "s2_sb")
        rs2 = T((2, 256), f32, "rs2")
        deg2 = T((2, 256), f32, "deg2")
        g2 = T((2, 256), f32, "g2")
        gt = T((2, 256), f32, "gt")
        diag_sb = T((2, 1024), f32, "diag_sb")
        M24 = T((24, 256), f32, "M24")
        tmp24 = T((24, 256), f32, "tmp24")
        mt = T((2, 256), f32, "mt")
        m2c = T((2, 256), f32, "m2c")
        conf2 = T((2, 1024), f32, "conf2")

        dma = nc.sync.dma_start

        # ---- input DMAs ----
        dma(f_sb[:].rearrange("p (t w) -> p t w", t=6),
            feat[:].rearrange("(t p) w -> p t w", p=128))
        dma(w1_sb[:].rearrange("p (c t h) -> p c t h", c=2, t=13),
            w1[:].rearrange("c (t p) h -> p c t h", p=128))
        dma(w2_sb[:].rearrange("p (c t h) -> p c t h", c=2, t=2),
            w2[:].rearrange("c (t p) h -> p c t h", p=128))
        dma(w3_sb[:], w3[:])
        dma(e_sb[:], e_in[:])
        dma(mw1_sb[:].rearrange("p (t h) -> p t h", t=6),
            mw1[:].rearrange("(t p) h -> p t h", p=128))
        dma(lw1_sb[:].rearrange("p (t h) -> p t h", t=6),
            lw1[:].rearrange("(t p) h -> p t h", p=128))
        dma(mw2_sb[:], mw2[:])
        dma(lw2_sb[:], lw2[:])
        dma(b1_sb[:], b1[:])
        dma(b2_sb[:], b2[:])
        dma(b3_sb[:], b3[:])
        dma(mb1_sb[:], mb1[:])
        dma(lb1_sb[:], lb1[:])
        dma(mb2_sb[:], mb2[:])
        dma(lb2_sb[:], lb2[:])
        dma(cam_sb[:], cam[:])
        dma(eps_sb[:], eps_in[:])
        dma(conf_sb[:], conf_in[:])
        dma(mask_sb[:], mask_in[:])
        dma(cma_sb[:], cma_in[:])
        dma(cmb_sb[:], cmb_in[:])
        m2_0 = pm.tile([2, 256], f32, tag="m2", name="m2_0")
        dma(m2_0[:].rearrange("c (i j) -> c i j", i=16), grid5(cam[:]))

        # ---- memsets (gpsimd is idle early) ----
        for c in range(2):
            for ht in range(2):
                nc.gpsimd.memset(x1[c][ht][:], 0.0)
        nc.gpsimd.memset(M24[:], 0.0)
        nc.gpsimd.memset(diag_sb[:], 1.0)

        # ---- feature prep: sampled points + spatial sum ----
        for dt in range(6):
            full = f_sb[:, dt * 1024:(dt + 1) * 1024]
            nc.vector.tensor_copy(
                fp_sb[:, dt * 256:(dt + 1) * 256].rearrange("p (i j) -> p i j", i=16),
                grid5(full))
            if dt < 3:
                nc.vector.reduce_sum(fsum[:, dt:dt + 1], full, axis=AX)
            else:
                nc.scalar.activation(scratch[:], full, AF.Copy,
                                     accum_out=fsum[:, dt:dt + 1])
        nc.vector.tensor_copy(fsum_bf[:], fsum[:])

        # ---- G matmuls: per-point projections ----
        def w1col(c, t, ht):
            base = (c * 13 + t) * 256 + ht * 128
            return w1_sb[:, base:base + 128]

        for c in range(2):
            for ht in range(2):
                for s in range(2):   # 0 = a-side, 1 = b-side
                    ps = pg.tile([128, 256], f32, tag="g", bufs=2, name=f"g_{c}_{ht}_{s}")
                    for dt in range(6):
                        nc.tensor.matmul(
                            ps[:], w1col(c, s * 6 + dt, ht),
                            fp_sb[:, dt * 256:(dt + 1) * 256],
                            start=(dt == 0), stop=False)
                    nc.tensor.matmul(ps[:], w1col(c, 12, ht), e_sb[:],
                                     start=False, stop=True)
                    if s == 0:
                        nc.scalar.activation(pa[c][ht][:], ps[:], AF.Identity,
                                             bias=b1_sb[:, c * 2 + ht:c * 2 + ht + 1])
                    else:
                        nc.vector.tensor_copy(pb[c][ht][:], ps[:])

        # ---- z path ----
        for w1s, w2s, b1s, b2s, h1s, zout in (
                (mw1_sb, mw2_sb, mb1_sb, mb2_sb, h1m, zmu_sb),
                (lw1_sb, lw2_sb, lb1_sb, lb2_sb, h1l, zls_sb)):
            for ht in range(2):
                psz = pzz.tile([128, 1], f32, tag="z1", bufs=1, name=f"z1_{ht}")
                for dt in range(6):
                    nc.tensor.matmul(psz[:],
                                     w1s[:, dt * 256 + ht * 128: dt * 256 + ht * 128 + 128],
                                     fsum_bf[:, dt:dt + 1],
                                     start=(dt == 0), stop=(dt == 5))
                nc.scalar.activation(h1s[:, ht:ht + 1], psz[:], AF.Relu,
                                     bias=b1s[:, ht:ht + 1])
            psz2 = pzz.tile([8, 1], f32, tag="z2", bufs=1, name="z2ps")
            for ht in range(2):
                nc.tensor.matmul(psz2[:], w2s[:, ht * 8:(ht + 1) * 8],
                                 h1s[:, ht:ht + 1], start=(ht == 0), stop=(ht == 1))
            nc.scalar.activation(zout[:], psz2[:], AF.Identity, bias=b2s[:])
        nc.scalar.activation(ez_sb[:], zls_sb[:], AF.Exp)
        nc.vector.scalar_tensor_tensor(zs_sb[:], ez_sb[:], eps_sb[:, 0:1], zmu_sb[:],
                                       op0=OP.mult, op1=OP.add)
        dma(z_o[0], zmu_sb[:, 0:1])
        dma(z_o[1], zls_sb[:, 0:1])
        dma(z_o[2], zs_sb[:, 0:1])

        # ---- pair assembly + relu ----
        cnt = 0
        for c in range(2):
            for ht in range(2):
                x1v = x1[c][ht][:].rearrange("p (o i j) -> p o i j", o=12, i=16)
                pav = pa[c][ht][:].rearrange("p (i j) -> p i j", i=16)
                pbv = pb[c][ht][:].rearrange("p (i j) -> p i j", i=16)
                for o, (di, dj) in enumerate(OFFS):
                    i0, i1, j0, j1 = _rect(di, dj)
                    eng = nc.vector if cnt % 2 == 0 else nc.gpsimd
                    eng.tensor_tensor(
                        x1v[:, o, i0:i1, j0:j1],
                        pav[:, i0:i1, j0:j1],
                        pbv[:, i0 + di:i1 + di, j0 + dj:j1 + dj],
                        op=OP.add)
                    cnt += 1
                nc.vector.tensor_relu(x1[c][ht][:], x1[c][ht][:])

        # ---- L2 ----
        for c in range(2):
            for grp in (range(0, 3), range(3, 6)):
                pstiles = {ch: px2.tile([128, 512], f32, tag="x2ps", bufs=3, name=f"x2ps_{c}_{ch}") for ch in grp}
                for ct in range(2):
                    lhs = w2_sb[:, (c * 2 + ct) * 128:(c * 2 + ct) * 128 + 128]
                    for ch in grp:
                        nc.tensor.matmul(pstiles[ch][:], lhs,
                                         x1[c][ct][:, ch * 512:(ch + 1) * 512],
                                         start=(ct == 0), stop=(ct == 1))
                for ch in grp:
                    nc.scalar.activation(x2[c][:, ch * 512:(ch + 1) * 512],
                                         pstiles[ch][:], AF.Relu,
                                         bias=b2_sb[:, c:c + 1])

        # ---- L3 + sigmoid ----
        for c in range(2):
            for ch in range(6):
                pl = pl3.tile([1, 512], f32, tag="l3", bufs=1, name=f"l3_{c}_{ch}")
                nc.tensor.matmul(pl[:], w3_sb[:, c:c + 1],
                                 x2[c][:, ch * 512:(ch + 1) * 512],
                                 start=True, stop=True)
                nc.scalar.activation(val_sb[:, c * 3072 + ch * 512: c * 3072 + (ch + 1) * 512],
                                     pl[:], AF.Sigmoid, bias=b3_sb[:, c:c + 1])

        # ---- spread val to 24 lanes: lane = 2*o + c ----
        for c in range(2):
            src_c = bass_rust.AP(val_sb[:].tensor, c * 3072,
                                 [[6144, 1], [256, 12], [1, 256]])
            dst_c = bass_rust.AP(val24[:].tensor, c * 256, [[512, 12], [1, 256]])
            dma(dst_c, src_c)

        # ---- normalize ----
        nc.vector.tensor_tensor(valm[:], val24[:], mask_sb[:], op=OP.mult)
        ps_s2 = ptail.tile([2, 256], f32, tag="x2ps", bufs=3, name="s2ps")
        nc.tensor.matmul(ps_s2[:], cma_sb[:], valm[:], start=True, stop=True)
        nc.vector.tensor_copy(s2_sb[:], ps_s2[:])
        nc.vector.tensor_scalar_add(gt[:], s2_sb[:], 1.0)
        nc.vector.reciprocal(rs2[:], gt[:])
        ps_rep = ptail.tile([24, 256], f32, tag="x2ps", bufs=3, name="repps")
        nc.tensor.matmul(ps_rep[:], cmb_sb[:], rs2[:], start=True, stop=True)
        nc.vector.tensor_tensor(valp[:], valm[:], ps_rep[:], op=OP.mult)

        # deg2 = (s2+1)*rs2 ; g2 = 1 + 0.1*(rs2 - deg2)
        nc.vector.scalar_tensor_tensor(deg2[:], s2_sb[:], 1.0, rs2[:],
                                       op0=OP.add, op1=OP.mult)
        nc.vector.tensor_tensor(gt[:], rs2[:], deg2[:], op=OP.subtract)
        nc.vector.tensor_scalar(g2[:], gt[:], 0.1, 1.0, op0=OP.mult, op1=OP.add)

        # ---- A scatter: diag + off-diag vals (via DRAM staging) ----
        nc.vector.tensor_copy(grid5(diag_sb[:]),
                              rs2[:].rearrange("c (i j) -> c i j", i=16))
        valp_d = pdram.tile([24, 256], f32, tag="valp_d", name="valp_d")
        diag_d = pdram.tile([2, 1024], f32, tag="diag_d", name="diag_d")
        dma(valp_d[:], valp[:])
        dma(diag_d[:], diag_sb[:])
        valp_dt = valp_d[:].tensor
        diag_dt = diag_d[:].tensor
        for c in range(2):
            dst = bass_rust.AP(A_flat_t, c * 1048576, [[1025, 1024]])
            dma(dst, bass_rust.AP(diag_dt, c * 1024, [[1, 1024]]))
        for o, (di, dj) in enumerate(OFFS):
            i0, i1, j0, j1 = _rect(di, dj)
            delta = 64 * di + 2 * dj
            for c in range(2):
                off = c * 1048576 + 1025 * (64 * i0 + 2 * j0) + delta
                dst = bass_rust.AP(A_flat_t, off,
                                   [[65600, i1 - i0], [2050, j1 - j0]])
                svp = bass_rust.AP(valp_dt, (2 * o + c) * 256 + i0 * 16 + j0,
                                   [[16, i1 - i0], [1, j1 - j0]])
                dma(dst, svp)

        # ---- diffusion: 3 steps on sampled points ----
        m_cur = m2_0
        for step in range(3):
            for o, (di, dj) in enumerate(OFFS):
                i0, i1, j0, j1 = _rect(di, dj)
                dma(M24[2 * o:2 * o + 2, :].rearrange("p (i j) -> p i j", i=16)[:, i0:i1, j0:j1],
                    m_cur[:].rearrange("c (i j) -> c i j", i=16)[:, i0 + di:i1 + di, j0 + dj:j1 + dj])
            nc.vector.tensor_tensor(tmp24[:], valp[:], M24[:], op=OP.mult)
            ps_n = ptail.tile([2, 256], f32, tag="x2ps", bufs=3, name=f"nps_{step}")
            nc.tensor.matmul(ps_n[:], cma_sb[:], tmp24[:], start=True, stop=True)
            nc.vector.tensor_tensor(mt[:], m_cur[:], g2[:], op=OP.mult)
            m_new = pm.tile([2, 256], f32, tag="m2", name=f"m2_{step}")
            nc.vector.scalar_tensor_tensor(m_new[:], ps_n[:], 0.1, mt[:],
                                           op0=OP.mult, op1=OP.add)
            m_cur = m_new

        # ---- outputs: cam_refined + confidence ----
        nc.vector.tensor_scalar(m2c[:], m_cur[:], 0.0, 1.0, op0=OP.max, op1=OP.min)
        nc.vector.tensor_copy(grid5(cam_sb[:]),
                              m2c[:].rearrange("c (i j) -> c i j", i=16))
        dma(cam_o[:], cam_sb[:])
        nc.scalar.activation(conf2[:], cam_sb[:], AF.Sigmoid,
                             bias=conf_sb[:, 1:2], scale=conf_sb[:, 0:1])
        dma(conf_o[:], conf2[:])

    nc.compile()
    _BUILD = nc
    return nc


def _prep_maps(inputs):
    mask, cma, cmb = _host_consts()
    spts = (np.arange(16)[:, None] * 64 + np.arange(16)[None, :] * 2).ravel()

    feats = np.asarray(inputs['features'], np.float32).reshape(B, D, HW)
    cam_raw = np.asarray(inputs['cam_raw'], np.float32).reshape(B, C, HW)
    eps = np.asarray(inputs['eps'], np.float32)
    ce = np.asarray(inputs['coord_emb'], np.float32)

    w1_bf = np.asarray(inputs['aff_W1'], np.float32).astype(BF16)
    w2_bf = np.asarray(inputs['aff_W2'], np.float32).astype(BF16)
    w3_bf = np.ascontiguousarray(np.asarray(inputs['aff_W3'], np.float32)[:, :, 0].T).astype(BF16)
    e_bf = np.ascontiguousarray((0.5 * ce[spts]).T).astype(BF16)
    mw1_bf = (np.asarray(inputs['mu_W1'], np.float32) / 1024.0).astype(BF16)
    lw1_bf = (np.asarray(inputs['ls_W1'], np.float32) / 1024.0).astype(BF16)
    mw2_bf = np.ascontiguousarray(
        np.asarray(inputs['mu_W2'], np.float32).reshape(2, 128, K).transpose(1, 0, 2).reshape(128, 16)).astype(BF16)
    lw2_bf = np.ascontiguousarray(
        np.asarray(inputs['ls_W2'], np.float32).reshape(2, 128, K).transpose(1, 0, 2).reshape(128, 16)).astype(BF16)
    b1_in = np.ascontiguousarray(
        np.asarray(inputs['aff_b1'], np.float32).reshape(2, 2, 128).transpose(2, 0, 1).reshape(128, 4))
    b2_in = np.ascontiguousarray(np.asarray(inputs['aff_b2'], np.float32).T)
    b3_in = np.ascontiguousarray(np.asarray(inputs['aff_b3'], np.float32).reshape(1, 2))
    mb1_in = np.ascontiguousarray(np.asarray(inputs['mu_b1'], np.float32).reshape(2, 128).T)
    lb1_in = np.ascontiguousarray(np.asarray(inputs['ls_b1'], np.float32).reshape(2, 128).T)
    mb2_in = np.asarray(inputs['mu_b2'], np.float32).reshape(8, 1)
    lb2_in = np.asarray(inputs['ls_b2'], np.float32).reshape(8, 1)
    conf_v = np.array([[float(np.asarray(inputs['conf_scale'])),
                        float(np.asarray(inputs['conf_bias']))]], np.float32)
    conf_in = np.repeat(conf_v, 2, axis=0)

    shared = dict(w1=w1_bf, w2=w2_bf, w3=w3_bf, e=e_bf, mw1=mw1_bf, lw1=lw1_bf,
                  mw2=mw2_bf, lw2=lw2_bf, b1=b1_in, b2=b2_in, b3=b3_in,
                  mb1=mb1_in, lb1=lb1_in, mb2=mb2_in, lb2=lb2_in,
                  conf=conf_in, mask=mask, cma=cma, cmb=cmb)
    maps = []
    for b in range(B):
        m = dict(shared)
        m['feat'] = feats[b].astype(BF16)
        m['cam'] = cam_raw[b]
        m['eps'] = eps[b].reshape(8, 1)
        maps.append(m)
    return maps


def _run(inputs, trace=False, trace_kwargs=None):
    from concourse.bass_utils import run_bass_kernel_spmd
    nc = _build()
    maps = _prep_maps(inputs)
    res = run_bass_kernel_spmd(nc, maps, core_ids=list(range(8)),
                               trace=trace, **(trace_kwargs or {}))
    outs = res.results
    cam_ref = np.stack([outs[b]['cam_o'] for b in range(B)]).reshape(B, C, H, W)
    z_mu = np.stack([outs[b]['z_o'][0] for b in range(B)])
    z_ls = np.stack([outs[b]['z_o'][1] for b in range(B)])
    z_s = np.stack([outs[b]['z_o'][2] for b in range(B)])
    A = np.stack([outs[b]['A'] for b in range(B)])
    conf = np.stack([outs[b]['conf_o'] for b in range(B)]).reshape(B, C, H, W)
    return (cam_ref, z_mu, z_ls, z_s, A, conf), res


def kernel(**inputs):
    out, _ = _run(inputs, trace=False)
    return out
